# revision 13
# baseline (speedup 1.0000x reference)
"""Causal self-attention (b=2, t=2048, d_model=1024, 16 heads) on 8 trn2 cores.

Sharding: tensor-parallel over heads (2 heads per core). Each core computes
qkv = x @ W_qkv[:, head-slice], attention for its heads, and a partial
out_heads @ W_proj[head-rows, :]. The 8 partial [4096, 1024] outputs are
summed on the host (the all-reduce after proj), plus b_proj.

Device layout notes:
- Host pre-transposes x to xT [1024, 4096] so the d_model contraction dim is
  on partitions for every matmul; no on-device input transposes are needed.
- Stage A computes Q^T/K^T/V^T = W.T @ xT with both heads stacked on the
  partition axis ([128] = 2 heads x 64 dims).
- V^T is PE-transposed back to V [rows, 2x(64+1)] with a ones column per head
  so the att@V matmul also accumulates the softmax denominator row.
- Scores are computed transposed (sT[k, q]); softmax needs no max-subtraction
  (logits ~ N(0,1), exp cannot overflow fp32).
- Causality: k-tiles above the diagonal are skipped, diagonal tiles compute
  only the valid column suffix, and one 128x128 upper-triangular mask
  multiply fixes the diagonal band.
- All matmuls run in float32r (full PE rate at N>=256, ~1e-4 relative error).
"""

import sys

sys.path.insert(0, "/opt/trn_rl_repo")

import numpy as np

import concourse.bass as bass  # noqa: F401
import concourse.tile as tile
from concourse import bacc, mybir

F32 = mybir.dt.float32
F32R = mybir.dt.float32r
BF16 = mybir.dt.bfloat16
DT_AT = BF16   # operand dtype for stage A matmuls and attention (B/C)
EXP = mybir.ActivationFunctionType.Exp
IDENT = mybir.ActivationFunctionType.Identity

B = 2
T = 2048
DM = 1024
NH = 16
HD = 64
ROWS = B * T            # 4096
NCORES = 8
HPC = NH // NCORES      # heads per core = 2
WCOLS = HPC * HD        # 128 qkv columns per core for each of q/k/v
QCH = 512               # query chunk
KTILE = 128             # key tile
NQC = T // QCH          # 4 query chunks per batch
NKT_B = T // KTILE      # 16 key tiles per batch
NRC = ROWS // QCH       # 8 row chunks
NKD = DM // 128         # 8 d_model k-tiles
VW = 2 * (HD + 1)       # 130: V block width (2 heads x (64 dims + ones col))


class _Alloc:
    """Tag-based routing to the right tile pool."""
    WORK = {"xt", "ea", "eb", "bc", "osb", "rc2"}
    WORK_BUFS = {"xt": 10, "ea": 4, "eb": 4, "bc": 2, "osb": 3, "rc2": 4}

    def __init__(self, pers, work, ps, pso):
        self.pers, self.work, self.ps, self.pso = pers, work, ps, pso

    def tile(self, shape, dt, tag):
        if tag in ("ps", "ps2"):
            return self.ps.tile(shape, dt, tag=tag, name=tag)
        if tag == "pso":
            return self.pso.tile(shape, dt, tag=tag, name=tag)
        if tag in self.WORK:
            return self.work.tile(shape, dt, tag=tag, name=tag, bufs=self.WORK_BUFS[tag])
        return self.pers.tile(shape, dt, tag=tag, name=tag)


def _emit_consts(nc, al, aps):
    (xt_d, wq_d, wk_d, wv_d, wp_d, bq_d, bk_d, bv_d, triu_d, e2_d, id_d,
     vones_d, out_d) = aps
    C = {}
    C["qt"] = al.tile([128, ROWS], DT_AT, tag="qt")
    C["kt"] = al.tile([128, ROWS], DT_AT, tag="kt")
    C["vt"] = al.tile([128, ROWS], DT_AT, tag="vt")
    C["v"] = al.tile([128, (ROWS // 128) * VW], DT_AT, tag="v")
    C["ot"] = al.tile([128, ROWS], F32R, tag="ot")
    C["wq"] = al.tile([128, DM], DT_AT, tag="wq")
    C["wk"] = al.tile([128, DM], DT_AT, tag="wk")
    C["wv"] = al.tile([128, DM], DT_AT, tag="wv")
    C["wp"] = al.tile([128, DM], F32R, tag="wp")
    C["bq"] = al.tile([128, 1], F32, tag="bq")
    C["bk"] = al.tile([128, 1], F32, tag="bk")
    C["bv"] = al.tile([128, 1], F32, tag="bv")
    C["triu"] = al.tile([128, 128], DT_AT, tag="triu")
    C["triu2"] = al.tile([128, 256], DT_AT, tag="triu2")
    C["e2"] = al.tile([2, 128], F32R, tag="e2")
    C["id"] = al.tile([128, 128], DT_AT, tag="id")
    for k in range(NKD):
        nc.sync.dma_start(C["wq"][:, k * 128:(k + 1) * 128], wq_d[k * 128:(k + 1) * 128, :])
        nc.sync.dma_start(C["wk"][:, k * 128:(k + 1) * 128], wk_d[k * 128:(k + 1) * 128, :])
        nc.sync.dma_start(C["wv"][:, k * 128:(k + 1) * 128], wv_d[k * 128:(k + 1) * 128, :])
    nc.sync.dma_start(C["wp"][:], wp_d[:])
    nc.sync.dma_start(C["bq"][:], bq_d[:])
    nc.sync.dma_start(C["bk"][:], bk_d[:])
    nc.sync.dma_start(C["bv"][:], bv_d[:])
    nc.sync.dma_start(C["triu"][:], triu_d[:])
    nc.sync.dma_start(C["triu2"][:, 0:128], triu_d[:])
    nc.sync.dma_start(C["triu2"][:, 128:256], triu_d[:])
    nc.sync.dma_start(C["e2"][:], e2_d[:])
    nc.sync.dma_start(C["id"][:], id_d[:])
    # ones columns of the V blocks (cols 64 and 129 of each 130-block)
    v_blocks = C["v"].rearrange("p (i w) -> p i w", w=VW)
    nc.sync.dma_start(v_blocks[:, :, HD], vones_d[:])
    nc.sync.dma_start(v_blocks[:, :, 2 * HD + 1], vones_d[:])
    return C


def _emit_body(nc, al, aps, C, parts=("a", "bc", "d")):
    (xt_d, wq_d, wk_d, wv_d, wp_d, bq_d, bk_d, bv_d, triu_d, e2_d, id_d,
     vones_d, out_d) = aps
    qt_sb, kt_sb, vt_sb, v_sb, ot_sb = C["qt"], C["kt"], C["vt"], C["v"], C["ot"]
    wq_sb, wk_sb, wv_sb, wp_sb = C["wq"], C["wk"], C["wv"], C["wp"]
    bq_sb, bk_sb, bv_sb = C["bq"], C["bk"], C["bv"]
    triu_sb, e2_sb, id_sb = C["triu"], C["e2"], C["id"]

    if "a" in parts:
        _emit_stage_a(nc, al, aps, C)
    if "bc" in parts:
        _emit_attn(nc, al, aps, C, do_d=("d" in parts))
    elif "d" in parts:
        _emit_proj_all(nc, al, aps, C)


def _emit_stage_a(nc, al, aps, C):
    (xt_d, wq_d, wk_d, wv_d, wp_d, bq_d, bk_d, bv_d, triu_d, e2_d, id_d,
     vones_d, out_d) = aps
    qt_sb, kt_sb, vt_sb, v_sb, ot_sb = C["qt"], C["kt"], C["vt"], C["v"], C["ot"]
    wq_sb, wk_sb, wv_sb, wp_sb = C["wq"], C["wk"], C["wv"], C["wp"]
    bq_sb, bk_sb, bv_sb = C["bq"], C["bk"], C["bv"]
    triu_sb, e2_sb, id_sb = C["triu"], C["e2"], C["id"]

    # ---- stage A: qkvT = W.T @ xT (+bias), then V^T -> V transposes ----
    for rc in range(NRC):
        cs = rc * QCH
        slot1 = al.tile([128, 2 * QCH], F32, tag="ps2")
        slot2 = al.tile([128, 2 * QCH], F32, tag="ps2")
        psq = slot1[:, 0:QCH]
        psk = slot1[:, QCH:2 * QCH]
        psv = slot2[:, 0:QCH]
        for k in range(NKD):
            xt_t = al.tile([128, QCH], DT_AT, tag="xt")
            nc.gpsimd.dma_start(xt_t[:], xt_d[k * 128:(k + 1) * 128, cs:cs + QCH])
            st = (k == 0)
            sp = (k == NKD - 1)
            nc.tensor.matmul(psq, wq_sb[:, k * 128:(k + 1) * 128], xt_t[:], start=st, stop=sp)
            nc.tensor.matmul(psk, wk_sb[:, k * 128:(k + 1) * 128], xt_t[:], start=st, stop=sp)
            nc.tensor.matmul(psv, wv_sb[:, k * 128:(k + 1) * 128], xt_t[:], start=st, stop=sp)
        nc.scalar.activation(qt_sb[:, cs:cs + QCH], psq, IDENT, bias=bq_sb[:])
        nc.scalar.activation(kt_sb[:, cs:cs + QCH], psk, IDENT, bias=bk_sb[:])
        nc.scalar.activation(vt_sb[:, cs:cs + QCH], psv, IDENT, bias=bv_sb[:])
        # transpose the 4 fresh 128-wide V^T blocks into V layout; the bf16
        # transpose outputs live in the second half of slot2
        for j in range(QCH // 128):
            i = rc * (QCH // 128) + j
            pst = slot2[:, QCH + 64 * j:QCH + 64 * (j + 1)].bitcast(DT_AT)
            nc.tensor.transpose(pst, vt_sb[:, i * 128:(i + 1) * 128], id_sb[:])
            nc.vector.tensor_copy(v_sb[:, i * VW:i * VW + HD], pst[:, 0:HD])
            nc.vector.tensor_copy(v_sb[:, i * VW + HD + 1:i * VW + 2 * HD + 1], pst[:, HD:128])


def _emit_proj_all(nc, al, aps, C):
    (xt_d, wq_d, wk_d, wv_d, wp_d, bq_d, bk_d, bv_d, triu_d, e2_d, id_d,
     vones_d, out_d) = aps
    ot_sb, wp_sb = C["ot"], C["wp"]
    for qt in range(ROWS // 128):
        q0 = qt * 128
        osb = al.tile([128, DM], F32, tag="osb")
        for ct in range(DM // 512):
            psp = al.tile([128, 512], F32, tag="ps")
            nc.tensor.matmul(psp[:], ot_sb[:, q0:q0 + 128],
                             wp_sb[:, ct * 512:(ct + 1) * 512])
            nc.vector.tensor_copy(osb[:, ct * 512:(ct + 1) * 512], psp[:])
        nc.sync.dma_start(out_d[q0:q0 + 128, :], osb[:])


def _emit_attn(nc, al, aps, C, do_d=True):
    (xt_d, wq_d, wk_d, wv_d, wp_d, bq_d, bk_d, bv_d, triu_d, e2_d, id_d,
     vones_d, out_d) = aps
    qt_sb, kt_sb, vt_sb, v_sb, ot_sb = C["qt"], C["kt"], C["vt"], C["v"], C["ot"]
    wq_sb, wk_sb, wv_sb, wp_sb = C["wq"], C["wk"], C["wv"], C["wp"]
    bq_sb, bk_sb, bv_sb = C["bq"], C["bk"], C["bv"]
    triu_sb, e2_sb, id_sb = C["triu"], C["e2"], C["id"]
    triu2_sb = C["triu2"]

    # ---- stages B/C/D: two query chunks interleaved per batch ----
    class Chunk:
        def __init__(self, b, qc):
            self.b, self.qc = b, qc
            self.qglob = b * T + qc * QCH
            self.nkt = (qc + 1) * (QCH // KTILE)
            self.pso2 = al.tile([HD + 1, 2 * QCH], F32, tag="pso")
            self.ps = {}

        def emit_B(self, kt):
            r = kt * KTILE - self.qc * QCH
            s = max(0, r)
            kcol = self.b * T + kt * KTILE
            ps2 = al.tile([128, 2 * QCH], F32, tag="ps2")
            nc.tensor.matmul(ps2[:, s:QCH], kt_sb[0:HD, kcol:kcol + KTILE],
                             qt_sb[0:HD, self.qglob + s:self.qglob + QCH])
            nc.tensor.matmul(ps2[:, QCH + s:], kt_sb[HD:128, kcol:kcol + KTILE],
                             qt_sb[HD:128, self.qglob + s:self.qglob + QCH])
            self.ps[kt] = ps2

        def emit_EC(self, kt):
            r = kt * KTILE - self.qc * QCH
            s = max(0, r)
            i = self.b * NKT_B + kt
            ps2 = self.ps.pop(kt)
            ea2 = al.tile([128, 2 * QCH], DT_AT, tag="ea")
            src_v = ps2.rearrange("p (h q) -> p h q", h=2)[:, :, s:]
            dst_v = ea2.rearrange("p (h q) -> p h q", h=2)[:, :, s:]
            nc.scalar.activation(dst_v, src_v, EXP, scale=0.125)
            if r >= 0:
                band = ea2.rearrange("p (h q) -> p h q", h=2)[:, :, s:s + KTILE]
                nc.vector.tensor_mul(band, band, triu2_sb[:].rearrange("p (h q) -> p h q", h=2))
            st = (kt == 0)
            sp = (kt == self.nkt - 1)
            pso_a = self.pso2[:, 0:QCH]
            pso_b = self.pso2[:, QCH:2 * QCH]
            nc.tensor.matmul(pso_a[:, s:], v_sb[:, i * VW:i * VW + HD + 1],
                             ea2[:, s:QCH], start=st, stop=sp)
            nc.tensor.matmul(pso_b[:, s:], v_sb[:, i * VW + HD + 1:i * VW + VW],
                             ea2[:, QCH + s:], start=st, stop=sp)

        def finalize(self):
            qglob = self.qglob
            pso_a = self.pso2[:, 0:QCH]
            pso_b = self.pso2[:, QCH:2 * QCH]
            rca = al.tile([1, QCH], F32R, tag="rc2")
            rcb = al.tile([1, QCH], F32R, tag="rc2")
            with nc.allow_low_precision(reason="f32r softmax denom recip"):
                nc.vector.reciprocal(rca[:], pso_a[HD:HD + 1, :])
                nc.vector.reciprocal(rcb[:], pso_b[HD:HD + 1, :])
            psbc = al.tile([128, 2 * QCH], F32, tag="ps2")
            nc.tensor.matmul(psbc[0:HD, 0:QCH], e2_sb[0:1, 0:HD], rca[:])
            nc.tensor.matmul(psbc[0:HD, QCH:2 * QCH], e2_sb[0:1, 0:HD], rcb[:])
            bc2 = al.tile([HD, 2 * QCH], F32, tag="bc")
            nc.vector.tensor_copy(bc2[:], psbc[0:HD, :])
            nc.vector.tensor_mul(ot_sb[0:HD, qglob:qglob + QCH], pso_a[0:HD, :], bc2[:, 0:QCH])
            nc.vector.tensor_mul(ot_sb[HD:128, qglob:qglob + QCH], pso_b[0:HD, :], bc2[:, QCH:])
            if do_d:
                for j in range(QCH // 128):
                    q0 = qglob + j * 128
                    osb = al.tile([128, DM], F32, tag="osb")
                    psp = al.tile([128, 2 * QCH], F32, tag="ps2")
                    for ct in range(DM // 512):
                        nc.tensor.matmul(psp[:, ct * 512:(ct + 1) * 512], ot_sb[:, q0:q0 + 128],
                                         wp_sb[:, ct * 512:(ct + 1) * 512])
                        nc.vector.tensor_copy(osb[:, ct * 512:(ct + 1) * 512],
                                              psp[:, ct * 512:(ct + 1) * 512])
                    nc.sync.dma_start(out_d[q0:q0 + 128, :], osb[:])

    for b in range(B):
        for qa, qb in ((0, 1), (2, 3)):
            cx = Chunk(b, qa)
            cy = Chunk(b, qb)
            # interleaved: both chunks alternate B / exp+AV per k-tile
            for kt in range(cx.nkt):
                cx.emit_B(kt)
                cy.emit_B(kt)
                cx.emit_EC(kt)
                cy.emit_EC(kt)
            cx.finalize()
            # solo tail of the longer chunk, software-pipelined depth 1
            if cy.nkt > cx.nkt:
                cy.emit_B(cx.nkt)
                for kt in range(cx.nkt, cy.nkt):
                    if kt + 1 < cy.nkt:
                        cy.emit_B(kt + 1)
                    cy.emit_EC(kt)
            cy.finalize()


def build_module(repeat=1, loop_n=0, parts=("a", "bc", "d"), pre_parts=()):
    nc = bacc.Bacc("TRN2", target_bir_lowering=False, debug=False,
                   enable_asserts=True, num_devices=NCORES)

    def din(name, shape, dt=F32R):
        return nc.dram_tensor(name, shape, dt, kind="ExternalInput").ap()

    aps = (
        din("xt", [DM, ROWS], DT_AT),
        din("wq", [DM, WCOLS], DT_AT),
        din("wk", [DM, WCOLS], DT_AT),
        din("wv", [DM, WCOLS], DT_AT),
        din("wp", [WCOLS, DM], F32R),
        din("bq", [WCOLS, 1], F32),
        din("bk", [WCOLS, 1], F32),
        din("bv", [WCOLS, 1], F32),
        din("triu", [128, 128], DT_AT),
        din("e2", [2, 128], F32R),
        din("ident", [128, 128], DT_AT),
        din("vones", [128, ROWS // 128], DT_AT),
        nc.dram_tensor("out", [ROWS, DM], F32, kind="ExternalOutput").ap(),
    )
    with tile.TileContext(nc) as tc:
        with tc.tile_pool(name="pers", bufs=1) as pers, \
             tc.tile_pool(name="work", bufs=4) as work, \
             tc.tile_pool(name="ps", bufs=2, space="PSUM") as psp, \
             tc.tile_pool(name="pso", bufs=2, space="PSUM") as psop:
            al = _Alloc(pers, work, psp, psop)
            consts = _emit_consts(nc, al, aps)
            if pre_parts:
                _emit_body(nc, al, aps, consts, parts=pre_parts)
            if loop_n:
                with tc.For_i(0, loop_n, 1):
                    _emit_body(nc, al, aps, consts, parts=parts)
            else:
                for r in range(repeat):
                    _emit_body(nc, al, aps, consts, parts=parts)
    nc.compile()
    return nc


def _host_prep(x, W_qkv, b_qkv, W_proj):
    import ml_dtypes
    bf16 = ml_dtypes.bfloat16
    x = np.asarray(x, np.float32)
    W_qkv = np.asarray(W_qkv, np.float32)
    b_qkv = np.asarray(b_qkv, np.float32)
    W_proj = np.asarray(W_proj, np.float32)
    xt = np.ascontiguousarray(x.reshape(ROWS, DM).T.astype(bf16))
    triu = np.triu(np.ones((128, 128), bf16))
    e2 = np.zeros((2, 128), np.float32)
    e2[0, 0:HD] = 1.0
    e2[1, HD:128] = 1.0
    ident = np.eye(128, dtype=bf16)
    in_maps = []
    for c in range(NCORES):
        h0 = c * WCOLS  # first qkv column of this core's 2 heads
        in_maps.append({
            "xt": xt,
            "wq": np.ascontiguousarray(W_qkv[:, h0:h0 + WCOLS].astype(bf16)),
            "wk": np.ascontiguousarray(W_qkv[:, DM + h0:DM + h0 + WCOLS].astype(bf16)),
            "wv": np.ascontiguousarray(W_qkv[:, 2 * DM + h0:2 * DM + h0 + WCOLS].astype(bf16)),
            "wp": np.ascontiguousarray(W_proj[h0:h0 + WCOLS, :]),
            "bq": np.ascontiguousarray(b_qkv[h0:h0 + WCOLS, None]),
            "bk": np.ascontiguousarray(b_qkv[DM + h0:DM + h0 + WCOLS, None]),
            "bv": np.ascontiguousarray(b_qkv[2 * DM + h0:2 * DM + h0 + WCOLS, None]),
            "triu": triu,
            "e2": e2,
            "ident": ident,
            "vones": np.ones((128, ROWS // 128), bf16),
        })
    return in_maps


class _Runner:
    """Compile once, execute many times (mirrors bass2jax.run_bass_via_pjrt)."""

    def __init__(self, nc):
        import jax
        from jax.sharding import Mesh, PartitionSpec
        from jax.experimental.shard_map import shard_map
        from concourse import bass2jax
        from concourse import mybir as _mybir

        bass2jax.install_neuronx_cc_hook()
        self.jax = jax
        in_names, out_names, out_avals, zero_shapes = [], [], [], []
        partition_name = nc.partition_id_tensor.name if nc.partition_id_tensor else None
        for alloc in nc.m.functions[0].allocations:
            if not isinstance(alloc, _mybir.MemoryLocationSet):
                continue
            name = alloc.memorylocations[0].name
            if alloc.kind == "ExternalInput":
                if name != partition_name:
                    in_names.append(name)
            elif alloc.kind == "ExternalOutput":
                shape = tuple(alloc.tensor_shape)
                dtype = _mybir.dt.np(alloc.dtype)
                out_names.append(name)
                out_avals.append(jax.core.ShapedArray(shape, dtype))
                zero_shapes.append((shape, dtype))
        self.in_names = in_names
        self.out_names = out_names
        self.out_avals = out_avals
        self.zero_shapes = zero_shapes
        n_params = len(in_names)
        n_outs = len(out_avals)
        all_in_names = in_names + out_names + ([partition_name] if partition_name else [])

        def _body(*args):
            operands = list(args)
            if partition_name is not None:
                operands.append(bass2jax.partition_id_tensor())
            outs = bass2jax._bass_exec_p.bind(
                *operands,
                out_avals=tuple(out_avals),
                in_names=tuple(all_in_names),
                out_names=tuple(out_names),
                lowering_input_output_aliases=(),
                sim_require_finite=True,
                sim_require_nnan=True,
                nc=nc,
            )
            return tuple(outs)

        devices = jax.devices()[:NCORES]
        mesh = Mesh(np.asarray(devices), ("core",))
        self.mesh = mesh
        self.pspec = PartitionSpec("core")
        in_specs = (PartitionSpec("core"),) * (n_params + n_outs)
        out_specs = (PartitionSpec("core"),) * n_outs
        self.donate = tuple(range(n_params, n_params + n_outs))
        self.sharded = jax.jit(
            shard_map(_body, mesh=mesh, in_specs=in_specs, out_specs=out_specs,
                      check_rep=False),
            donate_argnums=self.donate, keep_unused=True)

    def concat_inputs(self, in_maps):
        return [np.concatenate([np.asarray(m[name]) for m in in_maps], axis=0)
                for name in self.in_names]

    def zeros(self):
        return [np.zeros((NCORES * s[0], *s[1:]), d) for (s, d) in self.zero_shapes]

    def run(self, concat_in):
        outs = self.sharded(*concat_in, *self.zeros())
        outs = self.jax.block_until_ready(outs)
        return outs

    def device_inputs(self, concat_in):
        from jax.sharding import NamedSharding
        sh = NamedSharding(self.mesh, self.pspec)
        return [self.jax.device_put(a, sh) for a in concat_in]

    def device_zeros(self):
        import jax.numpy as jnp
        from jax.sharding import NamedSharding
        sh = NamedSharding(self.mesh, self.pspec)
        return [jnp.zeros((NCORES * s[0], *s[1:]), d, device=sh)
                for (s, d) in self.zero_shapes]

    def run_device(self, dev_in):
        outs = self.sharded(*dev_in, *self.device_zeros())
        outs = self.jax.block_until_ready(outs)
        return outs

    def split_out(self, outs):
        res = {}
        for i, name in enumerate(self.out_names):
            res[name] = np.asarray(outs[i]).reshape(NCORES, *self.out_avals[i].shape)
        return res


_CACHE = {}


def _get_runner(repeat=1, loop_n=0, parts=("a", "bc", "d"), pre_parts=()):
    key = ("runner", repeat, loop_n, tuple(parts), tuple(pre_parts))
    if key not in _CACHE:
        nc = build_module(repeat=repeat, loop_n=loop_n, parts=parts, pre_parts=pre_parts)
        _CACHE[key] = _Runner(nc)
    return _CACHE[key]


def kernel(x, W_qkv, b_qkv, W_proj, b_proj):
    runner = _get_runner(repeat=1)
    in_maps = _host_prep(x, W_qkv, b_qkv, W_proj)
    concat_in = runner.concat_inputs(in_maps)
    outs = runner.run(concat_in)
    parts = runner.split_out(outs)["out"]  # [8, 4096, 1024]
    full = parts.sum(axis=0, dtype=np.float64).astype(np.float32)
    full = full + np.asarray(b_proj, np.float32)[None, :]
    return full.reshape(B, T, DM)


# revision 14
# speedup vs baseline: 1.0992x; 1.0992x over previous
"""Causal self-attention (b=2, t=2048, d_model=1024, 16 heads) on 8 trn2 cores.

Sharding: tensor-parallel over heads (2 heads per core). Each core computes
qkv = x @ W_qkv[:, head-slice], attention for its heads, and a partial
out_heads @ W_proj[head-rows, :]. The 8 partial [4096, 1024] outputs are
summed on the host (the all-reduce after proj), plus b_proj.

Device layout notes:
- Host pre-transposes x to xT [1024, 4096] so the d_model contraction dim is
  on partitions for every matmul; no on-device input transposes are needed.
- Stage A computes Q^T/K^T/V^T = W.T @ xT with both heads stacked on the
  partition axis ([128] = 2 heads x 64 dims).
- V^T is PE-transposed back to V [rows, 2x(64+1)] with a ones column per head
  so the att@V matmul also accumulates the softmax denominator row.
- Scores are computed transposed (sT[k, q]); softmax needs no max-subtraction
  (logits ~ N(0,1), exp cannot overflow fp32).
- Causality: k-tiles above the diagonal are skipped, diagonal tiles compute
  only the valid column suffix, and one 128x128 upper-triangular mask
  multiply fixes the diagonal band.
- All matmuls run in float32r (full PE rate at N>=256, ~1e-4 relative error).
"""

import sys

sys.path.insert(0, "/opt/trn_rl_repo")

import numpy as np

import concourse.bass as bass  # noqa: F401
import concourse.tile as tile
from concourse import bacc, mybir

F32 = mybir.dt.float32
F32R = mybir.dt.float32r
BF16 = mybir.dt.bfloat16
DT_AT = BF16   # operand dtype for stage A matmuls and attention (B/C)
EXP = mybir.ActivationFunctionType.Exp
IDENT = mybir.ActivationFunctionType.Identity

B = 2
T = 2048
DM = 1024
NH = 16
HD = 64
ROWS = B * T            # 4096
NCORES = 8
HPC = NH // NCORES      # heads per core = 2
WCOLS = HPC * HD        # 128 qkv columns per core for each of q/k/v
QCH = 512               # query chunk
KTILE = 128             # key tile
NQC = T // QCH          # 4 query chunks per batch
NKT_B = T // KTILE      # 16 key tiles per batch
NRC = ROWS // QCH       # 8 row chunks
NKD = DM // 128         # 8 d_model k-tiles
VW = 2 * (HD + 1)       # 130: V block width (2 heads x (64 dims + ones col))


class _Alloc:
    """Tag-based routing to the right tile pool."""
    WORK = {"xt", "ea", "eb", "bc", "osb", "rc2"}
    WORK_BUFS = {"xt": 10, "ea": 6, "eb": 4, "bc": 2, "osb": 3, "rc2": 4}

    def __init__(self, pers, work, ps, pso):
        self.pers, self.work, self.ps, self.pso = pers, work, ps, pso

    def tile(self, shape, dt, tag):
        if tag in ("ps", "ps2"):
            return self.ps.tile(shape, dt, tag=tag, name=tag)
        if tag == "pso":
            return self.pso.tile(shape, dt, tag=tag, name=tag)
        if tag in self.WORK:
            return self.work.tile(shape, dt, tag=tag, name=tag, bufs=self.WORK_BUFS[tag])
        return self.pers.tile(shape, dt, tag=tag, name=tag)


def _emit_consts(nc, al, aps):
    (xt_d, wq_d, wk_d, wv_d, wp_d, bq_d, bk_d, bv_d, triu_d, e2_d, id_d,
     vones_d, out_d) = aps
    C = {}
    C["qt"] = al.tile([128, ROWS], DT_AT, tag="qt")
    C["kt"] = al.tile([128, ROWS], DT_AT, tag="kt")
    C["vt"] = al.tile([128, ROWS], DT_AT, tag="vt")
    C["v"] = al.tile([128, (ROWS // 128) * VW], DT_AT, tag="v")
    C["ot"] = al.tile([128, ROWS], F32R, tag="ot")
    C["wq"] = al.tile([128, DM], DT_AT, tag="wq")
    C["wk"] = al.tile([128, DM], DT_AT, tag="wk")
    C["wv"] = al.tile([128, DM], DT_AT, tag="wv")
    C["wp"] = al.tile([128, DM], F32R, tag="wp")
    C["bq"] = al.tile([128, 1], F32, tag="bq")
    C["bk"] = al.tile([128, 1], F32, tag="bk")
    C["bv"] = al.tile([128, 1], F32, tag="bv")
    C["triu"] = al.tile([128, 128], DT_AT, tag="triu")
    C["triu2"] = al.tile([128, 256], DT_AT, tag="triu2")
    C["e2"] = al.tile([2, 128], F32R, tag="e2")
    C["id"] = al.tile([128, 128], DT_AT, tag="id")
    for k in range(NKD):
        nc.sync.dma_start(C["wq"][:, k * 128:(k + 1) * 128], wq_d[k * 128:(k + 1) * 128, :])
        nc.sync.dma_start(C["wk"][:, k * 128:(k + 1) * 128], wk_d[k * 128:(k + 1) * 128, :])
        nc.sync.dma_start(C["wv"][:, k * 128:(k + 1) * 128], wv_d[k * 128:(k + 1) * 128, :])
    nc.sync.dma_start(C["wp"][:], wp_d[:])
    nc.sync.dma_start(C["bq"][:], bq_d[:])
    nc.sync.dma_start(C["bk"][:], bk_d[:])
    nc.sync.dma_start(C["bv"][:], bv_d[:])
    nc.sync.dma_start(C["triu"][:], triu_d[:])
    nc.sync.dma_start(C["triu2"][:, 0:128], triu_d[:])
    nc.sync.dma_start(C["triu2"][:, 128:256], triu_d[:])
    nc.sync.dma_start(C["e2"][:], e2_d[:])
    nc.sync.dma_start(C["id"][:], id_d[:])
    # ones columns of the V blocks (cols 64 and 129 of each 130-block)
    v_blocks = C["v"].rearrange("p (i w) -> p i w", w=VW)
    nc.sync.dma_start(v_blocks[:, :, HD], vones_d[:])
    nc.sync.dma_start(v_blocks[:, :, 2 * HD + 1], vones_d[:])
    return C


def _emit_body(nc, al, aps, C, parts=("a", "bc", "d")):
    (xt_d, wq_d, wk_d, wv_d, wp_d, bq_d, bk_d, bv_d, triu_d, e2_d, id_d,
     vones_d, out_d) = aps
    qt_sb, kt_sb, vt_sb, v_sb, ot_sb = C["qt"], C["kt"], C["vt"], C["v"], C["ot"]
    wq_sb, wk_sb, wv_sb, wp_sb = C["wq"], C["wk"], C["wv"], C["wp"]
    bq_sb, bk_sb, bv_sb = C["bq"], C["bk"], C["bv"]
    triu_sb, e2_sb, id_sb = C["triu"], C["e2"], C["id"]

    if "a" in parts:
        _emit_stage_a(nc, al, aps, C)
    if "bc" in parts:
        _emit_attn(nc, al, aps, C, do_d=("d" in parts))
    elif "d" in parts:
        _emit_proj_all(nc, al, aps, C)


def _emit_stage_a(nc, al, aps, C):
    (xt_d, wq_d, wk_d, wv_d, wp_d, bq_d, bk_d, bv_d, triu_d, e2_d, id_d,
     vones_d, out_d) = aps
    qt_sb, kt_sb, vt_sb, v_sb, ot_sb = C["qt"], C["kt"], C["vt"], C["v"], C["ot"]
    wq_sb, wk_sb, wv_sb, wp_sb = C["wq"], C["wk"], C["wv"], C["wp"]
    bq_sb, bk_sb, bv_sb = C["bq"], C["bk"], C["bv"]
    triu_sb, e2_sb, id_sb = C["triu"], C["e2"], C["id"]

    # ---- stage A: qkvT = W.T @ xT (+bias), then V^T -> V transposes ----
    for rc in range(NRC):
        cs = rc * QCH
        slot1 = al.tile([128, 2 * QCH], F32, tag="ps2")
        slot2 = al.tile([128, 2 * QCH], F32, tag="ps2")
        psq = slot1[:, 0:QCH]
        psk = slot1[:, QCH:2 * QCH]
        psv = slot2[:, 0:QCH]
        for k in range(NKD):
            xt_t = al.tile([128, QCH], DT_AT, tag="xt")
            nc.gpsimd.dma_start(xt_t[:], xt_d[k * 128:(k + 1) * 128, cs:cs + QCH])
            st = (k == 0)
            sp = (k == NKD - 1)
            nc.tensor.matmul(psq, wq_sb[:, k * 128:(k + 1) * 128], xt_t[:], start=st, stop=sp)
            nc.tensor.matmul(psk, wk_sb[:, k * 128:(k + 1) * 128], xt_t[:], start=st, stop=sp)
            nc.tensor.matmul(psv, wv_sb[:, k * 128:(k + 1) * 128], xt_t[:], start=st, stop=sp)
        nc.scalar.activation(qt_sb[:, cs:cs + QCH], psq, IDENT, bias=bq_sb[:])
        nc.scalar.activation(kt_sb[:, cs:cs + QCH], psk, IDENT, bias=bk_sb[:])
        nc.scalar.activation(vt_sb[:, cs:cs + QCH], psv, IDENT, bias=bv_sb[:])
        # transpose the 4 fresh 128-wide V^T blocks into V layout; the bf16
        # transpose outputs live in the second half of slot2
        for j in range(QCH // 128):
            i = rc * (QCH // 128) + j
            pst = slot2[:, QCH + 64 * j:QCH + 64 * (j + 1)].bitcast(DT_AT)
            nc.tensor.transpose(pst, vt_sb[:, i * 128:(i + 1) * 128], id_sb[:])
            nc.vector.tensor_copy(v_sb[:, i * VW:i * VW + HD], pst[:, 0:HD])
            nc.vector.tensor_copy(v_sb[:, i * VW + HD + 1:i * VW + 2 * HD + 1], pst[:, HD:128])


def _emit_proj_all(nc, al, aps, C):
    (xt_d, wq_d, wk_d, wv_d, wp_d, bq_d, bk_d, bv_d, triu_d, e2_d, id_d,
     vones_d, out_d) = aps
    ot_sb, wp_sb = C["ot"], C["wp"]
    for qt in range(ROWS // 128):
        q0 = qt * 128
        osb = al.tile([128, DM], F32, tag="osb")
        for ct in range(DM // 512):
            psp = al.tile([128, 512], F32, tag="ps")
            nc.tensor.matmul(psp[:], ot_sb[:, q0:q0 + 128],
                             wp_sb[:, ct * 512:(ct + 1) * 512])
            nc.vector.tensor_copy(osb[:, ct * 512:(ct + 1) * 512], psp[:])
        nc.sync.dma_start(out_d[q0:q0 + 128, :], osb[:])


def _emit_attn(nc, al, aps, C, do_d=True):
    (xt_d, wq_d, wk_d, wv_d, wp_d, bq_d, bk_d, bv_d, triu_d, e2_d, id_d,
     vones_d, out_d) = aps
    qt_sb, kt_sb, vt_sb, v_sb, ot_sb = C["qt"], C["kt"], C["vt"], C["v"], C["ot"]
    wq_sb, wk_sb, wv_sb, wp_sb = C["wq"], C["wk"], C["wv"], C["wp"]
    bq_sb, bk_sb, bv_sb = C["bq"], C["bk"], C["bv"]
    triu_sb, e2_sb, id_sb = C["triu"], C["e2"], C["id"]
    triu2_sb = C["triu2"]

    # ---- stages B/C/D per (batch, query chunk) ----
    # Score matmuls get a priority boost so the PE instruction stream keeps
    # ~2 k-tiles of scores in flight ahead of the exp->AV chain (the
    # scheduler's cost model under-prices exp, so without this PE convoys).
    tc = al.tc
    for b in range(B):
        for qc in range(NQC):
            qglob = b * T + qc * QCH
            nkt = (qc + 1) * (QCH // KTILE)
            pso2 = al.tile([HD + 1, 2 * QCH], F32, tag="pso")
            pso_a = pso2[:, 0:QCH]
            pso_b = pso2[:, QCH:2 * QCH]
            for kt in range(nkt):
                r = kt * KTILE - qc * QCH
                s = max(0, r)          # valid column suffix start
                i = b * NKT_B + kt     # global 128-row tile index for K/V
                kcol = b * T + kt * KTILE
                with tc.high_priority(offset=11):
                    ps2 = al.tile([128, 2 * QCH], F32, tag="ps2")
                    nc.tensor.matmul(ps2[:, s:QCH], kt_sb[0:HD, kcol:kcol + KTILE],
                                     qt_sb[0:HD, qglob + s:qglob + QCH])
                    nc.tensor.matmul(ps2[:, QCH + s:], kt_sb[HD:128, kcol:kcol + KTILE],
                                     qt_sb[HD:128, qglob + s:qglob + QCH])
                ea2 = al.tile([128, 2 * QCH], DT_AT, tag="ea")
                src_v = ps2.rearrange("p (h q) -> p h q", h=2)[:, :, s:]
                dst_v = ea2.rearrange("p (h q) -> p h q", h=2)[:, :, s:]
                nc.scalar.activation(dst_v, src_v, EXP, scale=0.125)
                if r >= 0:  # diagonal tile: triangular mask on the 128-col bands
                    band = ea2.rearrange("p (h q) -> p h q", h=2)[:, :, s:s + KTILE]
                    nc.vector.tensor_mul(band, band, triu2_sb[:].rearrange("p (h q) -> p h q", h=2))
                st = (kt == 0)
                sp = (kt == nkt - 1)
                nc.tensor.matmul(pso_a[:, s:], v_sb[:, i * VW:i * VW + HD + 1],
                                 ea2[:, s:QCH], start=st, stop=sp)
                nc.tensor.matmul(pso_b[:, s:], v_sb[:, i * VW + HD + 1:i * VW + VW],
                                 ea2[:, QCH + s:], start=st, stop=sp)
            # normalize by the accumulated denominator row (index HD)
            rca = al.tile([1, QCH], F32R, tag="rc2")
            rcb = al.tile([1, QCH], F32R, tag="rc2")
            with nc.allow_low_precision(reason="f32r softmax denom recip"):
                nc.vector.reciprocal(rca[:], pso_a[HD:HD + 1, :])
                nc.vector.reciprocal(rcb[:], pso_b[HD:HD + 1, :])
            psbc = al.tile([128, 2 * QCH], F32, tag="ps2")
            nc.tensor.matmul(psbc[0:HD, 0:QCH], e2_sb[0:1, 0:HD], rca[:])
            nc.tensor.matmul(psbc[0:HD, QCH:2 * QCH], e2_sb[0:1, 0:HD], rcb[:])
            bc2 = al.tile([HD, 2 * QCH], F32, tag="bc")
            nc.vector.tensor_copy(bc2[:], psbc[0:HD, :])
            nc.vector.tensor_mul(ot_sb[0:HD, qglob:qglob + QCH], pso_a[0:HD, :], bc2[:, 0:QCH])
            nc.vector.tensor_mul(ot_sb[HD:128, qglob:qglob + QCH], pso_b[0:HD, :], bc2[:, QCH:])
            # proj for this chunk's 4 query tiles
            for j in range(QCH // 128 if do_d else 0):
                q0 = qglob + j * 128
                osb = al.tile([128, DM], F32, tag="osb")
                psp = al.tile([128, 2 * QCH], F32, tag="ps2")
                for ct in range(DM // 512):
                    nc.tensor.matmul(psp[:, ct * 512:(ct + 1) * 512], ot_sb[:, q0:q0 + 128],
                                     wp_sb[:, ct * 512:(ct + 1) * 512])
                    nc.vector.tensor_copy(osb[:, ct * 512:(ct + 1) * 512],
                                          psp[:, ct * 512:(ct + 1) * 512])
                nc.sync.dma_start(out_d[q0:q0 + 128, :], osb[:])


def build_module(repeat=1, loop_n=0, parts=("a", "bc", "d"), pre_parts=()):
    nc = bacc.Bacc("TRN2", target_bir_lowering=False, debug=False,
                   enable_asserts=True, num_devices=NCORES)

    def din(name, shape, dt=F32R):
        return nc.dram_tensor(name, shape, dt, kind="ExternalInput").ap()

    aps = (
        din("xt", [DM, ROWS], DT_AT),
        din("wq", [DM, WCOLS], DT_AT),
        din("wk", [DM, WCOLS], DT_AT),
        din("wv", [DM, WCOLS], DT_AT),
        din("wp", [WCOLS, DM], F32R),
        din("bq", [WCOLS, 1], F32),
        din("bk", [WCOLS, 1], F32),
        din("bv", [WCOLS, 1], F32),
        din("triu", [128, 128], DT_AT),
        din("e2", [2, 128], F32R),
        din("ident", [128, 128], DT_AT),
        din("vones", [128, ROWS // 128], DT_AT),
        nc.dram_tensor("out", [ROWS, DM], F32, kind="ExternalOutput").ap(),
    )
    with tile.TileContext(nc) as tc:
        with tc.tile_pool(name="pers", bufs=1) as pers, \
             tc.tile_pool(name="work", bufs=4) as work, \
             tc.tile_pool(name="ps", bufs=3, space="PSUM") as psp, \
             tc.tile_pool(name="pso", bufs=1, space="PSUM") as psop:
            al = _Alloc(pers, work, psp, psop)
            al.tc = tc
            consts = _emit_consts(nc, al, aps)
            if pre_parts:
                _emit_body(nc, al, aps, consts, parts=pre_parts)
            if loop_n:
                with tc.For_i(0, loop_n, 1):
                    _emit_body(nc, al, aps, consts, parts=parts)
            else:
                for r in range(repeat):
                    _emit_body(nc, al, aps, consts, parts=parts)
    nc.compile()
    return nc


def _host_prep(x, W_qkv, b_qkv, W_proj):
    import ml_dtypes
    bf16 = ml_dtypes.bfloat16
    x = np.asarray(x, np.float32)
    W_qkv = np.asarray(W_qkv, np.float32)
    b_qkv = np.asarray(b_qkv, np.float32)
    W_proj = np.asarray(W_proj, np.float32)
    xt = np.ascontiguousarray(x.reshape(ROWS, DM).T.astype(bf16))
    triu = np.triu(np.ones((128, 128), bf16))
    e2 = np.zeros((2, 128), np.float32)
    e2[0, 0:HD] = 1.0
    e2[1, HD:128] = 1.0
    ident = np.eye(128, dtype=bf16)
    in_maps = []
    for c in range(NCORES):
        h0 = c * WCOLS  # first qkv column of this core's 2 heads
        in_maps.append({
            "xt": xt,
            "wq": np.ascontiguousarray(W_qkv[:, h0:h0 + WCOLS].astype(bf16)),
            "wk": np.ascontiguousarray(W_qkv[:, DM + h0:DM + h0 + WCOLS].astype(bf16)),
            "wv": np.ascontiguousarray(W_qkv[:, 2 * DM + h0:2 * DM + h0 + WCOLS].astype(bf16)),
            "wp": np.ascontiguousarray(W_proj[h0:h0 + WCOLS, :]),
            "bq": np.ascontiguousarray(b_qkv[h0:h0 + WCOLS, None]),
            "bk": np.ascontiguousarray(b_qkv[DM + h0:DM + h0 + WCOLS, None]),
            "bv": np.ascontiguousarray(b_qkv[2 * DM + h0:2 * DM + h0 + WCOLS, None]),
            "triu": triu,
            "e2": e2,
            "ident": ident,
            "vones": np.ones((128, ROWS // 128), bf16),
        })
    return in_maps


class _Runner:
    """Compile once, execute many times (mirrors bass2jax.run_bass_via_pjrt)."""

    def __init__(self, nc):
        import jax
        from jax.sharding import Mesh, PartitionSpec
        from jax.experimental.shard_map import shard_map
        from concourse import bass2jax
        from concourse import mybir as _mybir

        bass2jax.install_neuronx_cc_hook()
        self.jax = jax
        in_names, out_names, out_avals, zero_shapes = [], [], [], []
        partition_name = nc.partition_id_tensor.name if nc.partition_id_tensor else None
        for alloc in nc.m.functions[0].allocations:
            if not isinstance(alloc, _mybir.MemoryLocationSet):
                continue
            name = alloc.memorylocations[0].name
            if alloc.kind == "ExternalInput":
                if name != partition_name:
                    in_names.append(name)
            elif alloc.kind == "ExternalOutput":
                shape = tuple(alloc.tensor_shape)
                dtype = _mybir.dt.np(alloc.dtype)
                out_names.append(name)
                out_avals.append(jax.core.ShapedArray(shape, dtype))
                zero_shapes.append((shape, dtype))
        self.in_names = in_names
        self.out_names = out_names
        self.out_avals = out_avals
        self.zero_shapes = zero_shapes
        n_params = len(in_names)
        n_outs = len(out_avals)
        all_in_names = in_names + out_names + ([partition_name] if partition_name else [])

        def _body(*args):
            operands = list(args)
            if partition_name is not None:
                operands.append(bass2jax.partition_id_tensor())
            outs = bass2jax._bass_exec_p.bind(
                *operands,
                out_avals=tuple(out_avals),
                in_names=tuple(all_in_names),
                out_names=tuple(out_names),
                lowering_input_output_aliases=(),
                sim_require_finite=True,
                sim_require_nnan=True,
                nc=nc,
            )
            return tuple(outs)

        devices = jax.devices()[:NCORES]
        mesh = Mesh(np.asarray(devices), ("core",))
        self.mesh = mesh
        self.pspec = PartitionSpec("core")
        in_specs = (PartitionSpec("core"),) * (n_params + n_outs)
        out_specs = (PartitionSpec("core"),) * n_outs
        self.donate = tuple(range(n_params, n_params + n_outs))
        self.sharded = jax.jit(
            shard_map(_body, mesh=mesh, in_specs=in_specs, out_specs=out_specs,
                      check_rep=False),
            donate_argnums=self.donate, keep_unused=True)

    def concat_inputs(self, in_maps):
        return [np.concatenate([np.asarray(m[name]) for m in in_maps], axis=0)
                for name in self.in_names]

    def zeros(self):
        return [np.zeros((NCORES * s[0], *s[1:]), d) for (s, d) in self.zero_shapes]

    def run(self, concat_in):
        outs = self.sharded(*concat_in, *self.zeros())
        outs = self.jax.block_until_ready(outs)
        return outs

    def device_inputs(self, concat_in):
        from jax.sharding import NamedSharding
        sh = NamedSharding(self.mesh, self.pspec)
        return [self.jax.device_put(a, sh) for a in concat_in]

    def device_zeros(self):
        import jax.numpy as jnp
        from jax.sharding import NamedSharding
        sh = NamedSharding(self.mesh, self.pspec)
        return [jnp.zeros((NCORES * s[0], *s[1:]), d, device=sh)
                for (s, d) in self.zero_shapes]

    def run_device(self, dev_in):
        outs = self.sharded(*dev_in, *self.device_zeros())
        outs = self.jax.block_until_ready(outs)
        return outs

    def split_out(self, outs):
        res = {}
        for i, name in enumerate(self.out_names):
            res[name] = np.asarray(outs[i]).reshape(NCORES, *self.out_avals[i].shape)
        return res


_CACHE = {}


def _get_runner(repeat=1, loop_n=0, parts=("a", "bc", "d"), pre_parts=()):
    key = ("runner", repeat, loop_n, tuple(parts), tuple(pre_parts))
    if key not in _CACHE:
        nc = build_module(repeat=repeat, loop_n=loop_n, parts=parts, pre_parts=pre_parts)
        _CACHE[key] = _Runner(nc)
    return _CACHE[key]


def kernel(x, W_qkv, b_qkv, W_proj, b_proj):
    runner = _get_runner(repeat=1)
    in_maps = _host_prep(x, W_qkv, b_qkv, W_proj)
    concat_in = runner.concat_inputs(in_maps)
    outs = runner.run(concat_in)
    parts = runner.split_out(outs)["out"]  # [8, 4096, 1024]
    full = parts.sum(axis=0, dtype=np.float64).astype(np.float32)
    full = full + np.asarray(b_proj, np.float32)[None, :]
    return full.reshape(B, T, DM)
